# revision 1
# baseline (speedup 1.0000x reference)
"""Trainium2 Bass kernel for GQA attention (B=1,T=2048,D=3584,N=28,KH=4,H=128).

Sharding: 8 cores, one KV head per core-pair. Core c handles kv head c//2 and
query heads [7*(c//2)+4*(c%2) : ...] (4 on even cores, 3+1 dummy on odd cores,
padded to 4 so all cores run one SPMD program).

v2: fully-streamed single pass over four 512-column t-stripes. Per stripe:
  S1: per-unit QKV projection (bf16 matmuls, 28 D-chunks into one PSUM bank),
      bias via ACT, RoPE on DVE (bf16), v transposed to natural layout.
  S2: per query head: scoresT[s,t] = kT_chunk^T . qT (bf16), additive causal
      mask on the PSUM boundary region (DVE), exp via ACT -> P (bf16),
      denom = ones^T.P and attnT = v_chunk^T.P accumulated on PE,
      normalize with DVE reciprocal -> attnT (bf16).
  S3: y[t,:] += attnT^T . wo (bf16), PSUM drained via alternating ACT/DVE
      copies -> bf16 partial output, summed on host.
"""

import numpy as np
import ml_dtypes
from contextlib import ExitStack

import concourse.bass as bass
import concourse.bacc as bacc
import concourse.tile as tile
from concourse import mybir
from concourse.bass_utils import run_bass_kernel_spmd

F32 = mybir.dt.float32
BF16 = mybir.dt.bfloat16
FP8 = mybir.dt.float8e4

B, T, D = 1, 2048, 3584
N, KH, H = 28, 4, 128
G = N // KH              # 7 query heads per kv head
NQ = 4                   # padded query heads per core
NU = NQ + 2              # q0..q3, k, v projection units
DC = D // 128            # 28 contraction chunks
TS = 512                 # moving-dim tile (t stripes)
NT = T // TS             # 4
SCC = T // 128           # 16 key chunks
SCALE = float(H) ** -0.5
MASKVAL = -30000.0       # pre-exp additive mask; exp((s+m)*SCALE) -> 0

_TRACE = False           # test.py flips this to get an NTFF profile


def build_program():
    nc = bacc.Bacc(None)
    _build_body(nc)
    nc.compile()
    return nc


def _build_body(nc):
    xT_d = nc.dram_tensor("xT16", [NT, 4, 128, DC // 4, TS], BF16,
                          kind="ExternalInput")
    wqk_d = nc.dram_tensor("wqk16", [4, 128, DC // 4, NU * 128], BF16,
                           kind="ExternalInput")
    bias_d = nc.dram_tensor("biasT", [128, NU], F32, kind="ExternalInput")
    cos_d = nc.dram_tensor("cosT", [128, T], BF16, kind="ExternalInput")
    sin_d = nc.dram_tensor("sinT", [128, T], BF16, kind="ExternalInput")
    ident_d = nc.dram_tensor("ident", [128, 128], F32, kind="ExternalInput")
    mask_d = nc.dram_tensor("maskP", [NQ, 128, TS], BF16, kind="ExternalInput")
    wo_d = nc.dram_tensor("woT", [128, NQ, D], BF16, kind="ExternalInput")
    ones_d = nc.dram_tensor("ones", [128, 128], BF16, kind="ExternalInput")
    y_d = nc.dram_tensor("y", [T, D], BF16, kind="ExternalOutput")

    with tile.TileContext(nc) as tc, ExitStack() as ctx:
        persist = ctx.enter_context(tc.tile_pool(name="persist", bufs=1))

        qkT = [persist.tile([128, T], BF16, tag=f"qkT{u}", name=f"qkT{u}")
               for u in range(NQ + 1)]
        v_nat = persist.tile([128, SCC, 128], BF16, tag="vnat", name="vnat")
        vT_sb = persist.tile([128, T], F32, tag="vT")
        attnT = [persist.tile([128, T], BF16, tag=f"attnT{u}", name=f"attnT{u}")
                 for u in range(NQ)]
        cos_sb = persist.tile([128, T], BF16, tag="cos")
        sin_sb = persist.tile([128, T], BF16, tag="sin")
        bias_sb = persist.tile([128, NU], F32, tag="bias")
        ident_sb = persist.tile([128, 128], F32, tag="ident")
        ones_sb = persist.tile([128, 128], BF16, tag="ones")
        mask_sb = persist.tile([128, NQ, TS], BF16, tag="mask")
        wqk_sb = [persist.tile([128, DC // 4, NU * 128], BF16,
                               tag=f"wqkg{g}", name=f"wqkg{g}")
                  for g in range(4)]
        wo_sb = persist.tile([128, NQ, D], BF16, tag="wo")

        xpool = ctx.enter_context(tc.tile_pool(name="xp", bufs=6))
        tpool = ctx.enter_context(tc.tile_pool(name="tp", bufs=3))
        spool = ctx.enter_context(tc.tile_pool(name="sp", bufs=2))
        ppool = ctx.enter_context(tc.tile_pool(name="pp", bufs=6))
        rpool = ctx.enter_context(tc.tile_pool(name="rp", bufs=2))
        ypool = ctx.enter_context(tc.tile_pool(name="yp", bufs=2))
        ps = ctx.enter_context(tc.tile_pool(name="ps", bufs=1, space="PSUM"))

        x_sb = [None] * NT

        def load_x(ts):
            # four separate quarter-tiles: the first S1 matmuls only wait on
            # the first ~0.9MB of the stripe, not the whole 3.7MB transfer.
            x_sb[ts] = [xpool.tile([128, DC // 4, TS], BF16, tag="x",
                                   name=f"x{ts}q{j}") for j in range(4)]
            for j in range(4):
                nc.sync.dma_start(x_sb[ts][j][:], xT_d[ts, j])

        # first weight group, then stripe-0 x, then the rest: the PE can
        # start after ~2.3MB of DMA and two descriptors.
        nc.sync.dma_start(wqk_sb[0][:], wqk_d[0])
        load_x(0)
        for g in range(1, 4):
            nc.sync.dma_start(wqk_sb[g][:], wqk_d[g])
            if g == 1:
                nc.sync.dma_start(cos_sb[:], cos_d[:])
                nc.sync.dma_start(sin_sb[:], sin_d[:])
                nc.sync.dma_start(bias_sb[:], bias_d[:])
                nc.sync.dma_start(ident_sb[:], ident_d[:])
                nc.sync.dma_start(ones_sb[:], ones_d[:])
                for k in range(NQ):
                    nc.sync.dma_start(mask_sb[:, k, :], mask_d[k])
                warm = tpool.tile([128, 1], F32, tag="warm", name="warm")
                nc.scalar.activation(warm[:], bias_sb[:, 0:1],
                                     mybir.ActivationFunctionType.Exp,
                                     scale=1.0)
        for u in range(NQ):
            nc.sync.dma_start(wo_sb[:, u, :], wo_d[:, u, :])

        for ts in range(NT):
            tsl = slice(ts * TS, (ts + 1) * TS)
            if ts + 1 < NT:
                load_x(ts + 1)

            # ---------------- S1: projections + RoPE ----------------
            # units in pairs on alternating PSUM banks: consecutive matmuls
            # are independent so the PE hides each LDWEIGHTS under the
            # previous matmul. k/v pair first so attention deps clear early.
            def drain_unit(u, up):
                if u == NU - 1:  # v: bias only, stays un-rotated
                    nc.scalar.activation(
                        vT_sb[:, tsl], up[:],
                        mybir.ActivationFunctionType.Identity,
                        bias=bias_sb[:, u:u + 1])
                    return
                tmp = tpool.tile([128, TS], BF16, tag="ropein", name="tmp")
                nc.scalar.activation(
                    tmp[:], up[:],
                    mybir.ActivationFunctionType.Identity,
                    bias=bias_sb[:, u:u + 1])
                # RoPE with half-swap routed through full-height scratches
                # (DVE needs equal base partitions on both SB inputs;
                # outputs may differ). cos2=[c;c], sin2=[-s;+s] host-made.
                dst = qkT[u][:, tsl]
                c2, s2 = cos_sb[:, tsl], sin_sb[:, tsl]
                scrA = spool.tile([128, TS], BF16, tag="ropescrA", name="scrA")
                scrB = spool.tile([128, TS], BF16, tag="ropescrB", name="scrB")
                nc.vector.tensor_mul(scrA[0:64, :], tmp[64:128, :],
                                     s2[64:128, :])
                nc.vector.tensor_mul(scrA[64:128, :], tmp[0:64, :],
                                     s2[0:64, :])
                nc.vector.tensor_mul(scrB[:], tmp[:], c2)
                nc.vector.tensor_add(dst, scrB[:], scrA[:])

            ups = [ps.tile([128, TS], F32, tag="mm6", bufs=6, name=f"up{u}")
                   for u in range(NU)]
            for dc in range(DC):
                for u in range(NU):
                    nc.tensor.matmul(
                        ups[u][:],
                        wqk_sb[dc // 7][:, dc % 7, u * 128:(u + 1) * 128],
                        x_sb[ts][dc // 7][:, dc % 7, :],
                        start=(dc == 0), stop=(dc == DC - 1))
            for u in (4, 5, 0, 1, 2, 3):
                drain_unit(u, ups[u])

            # v: [H, s] -> natural [s, H] (bf16) via PE transpose
            for i, sc in enumerate(range(4 * ts, 4 * ts + 4)):
                vp = ps.tile([128, TS], F32, tag=("den" if i % 2 == 0
                                                  else "av"), bufs=1,
                             name="vp")
                nc.tensor.transpose(
                    vp[:, 0:128], vT_sb[:, sc * 128:(sc + 1) * 128],
                    ident_sb[:])
                nc.scalar.copy(v_nat[:, sc, :], vp[:, 0:128])

            # ---------------- S2: attention cells ----------------
            kT = qkT[NQ]
            nsc = 4 * (ts + 1)
            cells = [(hq, sc) for hq in range(NQ) for sc in range(nsc)]
            pts = [None] * len(cells)

            def issue_scores(i):
                hq, sc = cells[i]
                sc_ps = ps.tile([128, TS], F32, tag="mm6", bufs=6,
                                name="scps")
                nc.tensor.matmul(
                    sc_ps[:],
                    kT[:, sc * 128:(sc + 1) * 128],
                    qkT[hq][:, tsl],
                    start=True, stop=True)
                k = sc - 4 * ts
                if k >= 0:
                    w = 128 * (k + 1)
                    nc.vector.tensor_add(
                        sc_ps[:, 0:w], sc_ps[:, 0:w], mask_sb[:, k, 0:w])
                pt = ppool.tile([128, TS], BF16, tag="pt", name="pt")
                nc.scalar.activation(
                    pt[:], sc_ps[:],
                    mybir.ActivationFunctionType.Exp, scale=SCALE)
                pts[i] = pt

            # one pipeline across all cells of the stripe: scores run a few
            # chunks ahead of den/av (across head boundaries too) so the PE
            # never waits on the current chunk's exp or a cell transition.
            LA = 5
            den_ps = av_ps = None
            for i0 in range(min(LA, len(cells))):
                issue_scores(i0)
            for i, (hq, sc) in enumerate(cells):
                if i + LA < len(cells):
                    issue_scores(i + LA)
                if sc == 0:
                    den_ps = ps.tile([128, TS], F32, tag="den", bufs=1,
                                     name="den")
                    av_ps = ps.tile([128, TS], F32, tag="av", bufs=1,
                                    name="av")
                st, sp = (sc == 0), (sc == nsc - 1)
                pt = pts[i]
                nc.tensor.matmul(den_ps[:], ones_sb[:], pt[:],
                                 start=st, stop=sp)
                nc.tensor.matmul(av_ps[:], v_nat[:, sc, :], pt[:],
                                 start=st, stop=sp)
                pts[i] = None
                if sp:
                    recip = rpool.tile([128, TS], F32, tag="recip",
                                       name="recip")
                    nc.vector.reciprocal_approx_fast(recip[:], den_ps[:])
                    nc.vector.tensor_mul(attnT[hq][:, tsl], av_ps[:],
                                         recip[:])

            # adjacent nt-tiles accumulate on alternating banks so
            # consecutive matmuls are independent (LDWEIGHTS hides) and the
            # stationary attnT block is reused by the pair.
            for tb in range(4 * ts, 4 * ts + 4):
                tbl = slice(tb * 128, (tb + 1) * 128)
                y_row = ypool.tile([128, D], BF16, tag="yrow", name="yrow")
                for nt0 in range(0, D // TS, 2):
                    pair = [nt0] if nt0 + 1 >= D // TS else [nt0, nt0 + 1]
                    yps = [ps.tile([128, TS], F32, tag="mm6", bufs=6,
                                   name=f"yp{j}") for j in range(len(pair))]
                    for u in range(NQ):
                        for j, nt in enumerate(pair):
                            nsl = slice(nt * TS, (nt + 1) * TS)
                            nc.tensor.matmul(
                                yps[j][:], attnT[u][:, tbl],
                                wo_sb[:, u, nsl],
                                start=(u == 0), stop=(u == NQ - 1))
                    for j, nt in enumerate(pair):
                        nsl = slice(nt * TS, (nt + 1) * TS)
                        if (tb + nt) % 2 == 0:
                            nc.scalar.copy(y_row[:, nsl], yps[j][:])
                        else:
                            nc.vector.tensor_copy(y_row[:, nsl], yps[j][:])
                nc.sync.dma_start(y_d[tbl, :], y_row[:])


def kernel(x, attn_mask, sin, cos, wq, wk, wv, wo, q_bias, k_bias, v_bias):
    x = np.asarray(x, np.float32)
    mask = np.asarray(attn_mask).astype(bool)
    sin = np.asarray(sin, np.float32)
    cos = np.asarray(cos, np.float32)
    wq = np.asarray(wq, np.float32)
    wk = np.asarray(wk, np.float32)
    wv = np.asarray(wv, np.float32)
    wo = np.asarray(wo, np.float32)
    q_bias = np.asarray(q_bias, np.float32).reshape(N, H)
    k_bias = np.asarray(k_bias, np.float32).reshape(KH, H)
    v_bias = np.asarray(v_bias, np.float32).reshape(KH, H)

    # causal-mask sanity: the kernel hardcodes the causal structure
    assert mask[0, 10, :11].all() and not mask[0, 10, 11:].any()

    BF = ml_dtypes.bfloat16
    xT = np.ascontiguousarray(x[0].T)                        # [D, T]
    xT16 = np.ascontiguousarray(
        xT.reshape(4, DC // 4, 128, NT, TS).transpose(3, 0, 2, 1, 4)
    ).astype(BF)
    c = cos[0].T                                             # [64, T]
    s = sin[0].T
    cosT = np.ascontiguousarray(np.concatenate([c, c], 0)).astype(BF)
    sinT = np.ascontiguousarray(np.concatenate([s, -s], 0)).astype(BF)
    ident = np.eye(128, dtype=np.float32)
    ones128 = np.ones((128, 128), BF)

    # additive pre-exp masks for boundary chunk k: invalid iff j < 128k + p
    p = np.arange(128)[:, None]
    j = np.arange(TS)[None, :]
    maskP = np.stack([np.where(j < 128 * k + p, MASKVAL, 0.0)
                      for k in range(NQ)]).astype(BF)        # [4,128,TS]

    in_maps = []
    for cix in range(8):
        kv = cix // 2
        qh = list(range(7 * kv + 4 * (cix % 2),
                        7 * kv + (4 if cix % 2 == 0 else 7)))
        cols = []    # [D, 128] per unit
        bcols = []   # [128] per unit
        for slot in range(NQ):
            if slot < len(qh):
                cols.append(wq[:, qh[slot], :])
                bcols.append(q_bias[qh[slot]])
            else:
                cols.append(np.zeros((D, H), np.float32))
                bcols.append(np.zeros(H, np.float32))
        cols += [wk[:, kv, :], wv[:, kv, :]]
        bcols += [k_bias[kv], v_bias[kv]]
        wqk = np.concatenate(cols, axis=1).reshape(
            4, DC // 4, 128, NU * 128).transpose(0, 2, 1, 3)
        biasT = np.stack(bcols, axis=1)                      # [128, NU]
        wo_rows = [wo[qh[sl]] if sl < len(qh) else np.zeros((H, D), np.float32)
                   for sl in range(NQ)]
        woT = np.stack(wo_rows, axis=1).astype(BF)           # [128, NQ, D]
        in_maps.append({
            "xT16": xT16, "wqk16": np.ascontiguousarray(wqk).astype(BF),
            "biasT": biasT, "cosT": cosT, "sinT": sinT, "ident": ident,
            "maskP": maskP, "woT": np.ascontiguousarray(woT),
            "ones": ones128,
        })

    nc = build_program()
    res = run_bass_kernel_spmd(nc, in_maps, list(range(8)), trace=_TRACE)
    if _TRACE and res.exec_time_ns is not None:
        print(f"HW exec time: {res.exec_time_ns} ns")
    y = np.zeros((T, D), np.float64)
    for r in res.results:
        y += r["y"].astype(np.float64)
    return y.reshape(B, T, D).astype(np.float32)



# revision 4
# speedup vs baseline: 1.2062x; 1.2062x over previous
"""Trainium2 Bass kernel for GQA attention (B=1,T=2048,D=3584,N=28,KH=4,H=128).

v3 sharding: heads x sequence 2D split over 8 cores.
  Pair p = cores (2p, 2p+1) owns kv head p and query heads 7p..7p+6.
  Within a pair, lane l = core % 2 owns the even (l=0) / odd (l=1)
  128-token query blocks -- the even/odd interleave is the optimal
  balanced causal split (both lanes sum to 68 key-chunk visits, and the
  SPMD union profile ceil((16-sc)/2) adds only 4 phantom chunk-columns).

Per core:
  S1: 9 projection units (7 q heads + k + v) x 28 D-chunks over my 1024
      tokens. K is roped, V transposed to natural layout; both go into a
      pairwise DRAM AllGather so each core gets K/V for all 2048 keys
      while the PE streams the Q projections (collective fully hidden).
  S2: per (head, key chunk): scoresT = kT_chunk^T qT over the union
      suffix of my query blocks >= chunk, one data-driven mask add per
      chunk (tri/full/zero content from host), exp on ACT, den/av
      accumulated on alternating PSUM banks, DVE normalize -> attnT
      (aliased onto the qT tile).
  S3: y[my tokens, :] = sum_h attnT_h^T wo_h, wo streamed per 512-col
      tile, drains alternating ACT/DVE, host sums the 4 head-group
      partials per token row.
"""

import numpy as np
import ml_dtypes
from contextlib import ExitStack

import concourse.bass as bass
import concourse.bacc as bacc
import concourse.tile as tile
from concourse import mybir
from concourse.bass_utils import run_bass_kernel_spmd

F32 = mybir.dt.float32
BF16 = mybir.dt.bfloat16

B, T, D = 1, 2048, 3584
N, KH, H = 28, 4, 128
NQ = 7                   # query heads per core
NU = NQ + 2              # + k, v units
DC = D // 128            # 28 contraction chunks
TS = 512
NB = 16                  # 128-token blocks in T
MYB = 8                  # my blocks per core
SCALE = float(H) ** -0.5
MASKVAL = -30000.0

# union suffix profile: #active 128-col blocks at key chunk sc, per stripe
W_ST = {
    0: [4, 4, 3, 3, 2, 2, 1, 1],
    1: [4, 4, 4, 4, 4, 4, 4, 4, 4, 4, 3, 3, 2, 2, 1, 1],
}

_TRACE = False           # test.py flips this to get an NTFF profile


def build_program():
    nc = bacc.Bacc(None)
    _build_body(nc)
    nc.compile()
    return nc


def _build_body(nc):
    xT_d = nc.dram_tensor("xT16", [2, 128, DC, TS], BF16, kind="ExternalInput")
    wqk_d = nc.dram_tensor("wqk16", [128, DC, NU, 128], BF16,
                           kind="ExternalInput")
    bias_d = nc.dram_tensor("biasT", [128, NU], F32, kind="ExternalInput")
    cos_d = nc.dram_tensor("cosT", [128, 1024], BF16, kind="ExternalInput")
    sin_d = nc.dram_tensor("sinT", [128, 1024], BF16, kind="ExternalInput")
    ident_d = nc.dram_tensor("ident", [128, 128], F32, kind="ExternalInput")
    mask_d = nc.dram_tensor("maskC", [128, NB, 128], BF16,
                            kind="ExternalInput")
    wo_d = nc.dram_tensor("woT", [128, NQ, D], BF16, kind="ExternalInput")
    ones_d = nc.dram_tensor("ones", [128, 128], BF16, kind="ExternalInput")
    y_d = nc.dram_tensor("y", [1024, D], BF16, kind="ExternalOutput")

    with tile.TileContext(nc) as tc, ExitStack() as ctx:
        persist = ctx.enter_context(tc.tile_pool(name="persist", bufs=1))

        wqk_sb = persist.tile([128, DC, NU, 128], BF16, tag="wqk")
        x_sb = [persist.tile([128, DC, TS], BF16, tag=f"x{st}",
                             name=f"x{st}")
                for st in (0, 1)]
        # qaT[u]: S1 writes roped qT here; S2 drains overwrite it with attnT
        # (last q read of a head-stripe precedes its attnT write).
        qaT = [persist.tile([128, 1024], BF16, tag=f"qaT{u}",
                            name=f"qaT{u}")
               for u in range(NQ)]
        blob = persist.tile([128, 2048], BF16, tag="blob")   # kT|v_nat mine
        kT_full = persist.tile([128, NB, 128], BF16, tag="ktf")
        v_nat = persist.tile([128, NB, 128], BF16, tag="vnat")
        cos_sb = persist.tile([128, 1024], BF16, tag="cos")
        sin_sb = persist.tile([128, 1024], BF16, tag="sin")
        bias_sb = persist.tile([128, NU], F32, tag="bias")
        ident_sb = persist.tile([128, 128], F32, tag="ident")
        ones_sb = persist.tile([128, 128], BF16, tag="ones")
        mask_sb = persist.tile([128, NB, 128], BF16, tag="mask")

        tpool = ctx.enter_context(tc.tile_pool(name="tp", bufs=3))
        spool = ctx.enter_context(tc.tile_pool(name="sp", bufs=2))
        ppool = ctx.enter_context(tc.tile_pool(name="pp", bufs=5))
        rpool = ctx.enter_context(tc.tile_pool(name="rp", bufs=2))
        wpool = ctx.enter_context(tc.tile_pool(name="wp", bufs=3))
        ypool = ctx.enter_context(tc.tile_pool(name="yp", bufs=4))
        ps = ctx.enter_context(tc.tile_pool(name="ps", bufs=1, space="PSUM"))
        dram = ctx.enter_context(tc.tile_pool(name="dram", bufs=1,
                                              space="DRAM"))

        in_b = dram.tile([128, 2048], BF16, tag="inb")
        out_b = dram.tile([2, 128, NB, 128], BF16, tag="outb")

        # ---------------- loads (staggered for early PE start) ----------
        for g in range(4):
            gs = slice(7 * g, 7 * g + 7)
            nc.sync.dma_start(wqk_sb[:, gs, :, :], wqk_d[:, gs, :, :])
            for st in (0, 1):
                nc.sync.dma_start(x_sb[st][:, gs, :], xT_d[st, :, gs, :])
            if g == 1:
                nc.sync.dma_start(cos_sb[:], cos_d[:])
                nc.sync.dma_start(sin_sb[:], sin_d[:])
                nc.sync.dma_start(bias_sb[:], bias_d[:])
                nc.sync.dma_start(ident_sb[:], ident_d[:])
                nc.sync.dma_start(ones_sb[:], ones_d[:])
                nc.sync.dma_start(mask_sb[:], mask_d[:])
                warm = tpool.tile([128, 1], F32, tag="warm", name="warm")
                nc.scalar.activation(warm[:], bias_sb[:, 0:1],
                                     mybir.ActivationFunctionType.Exp,
                                     scale=1.0)

        def rope_drain(up, u, st, dst):
            # dst = rope(psum + bias): [x1 c - x2 s ; x2 c + x1 s]
            tmp = tpool.tile([128, TS], BF16, tag="tmp", name="tmp")
            nc.scalar.activation(tmp[:], up[:],
                                 mybir.ActivationFunctionType.Identity,
                                 bias=bias_sb[:, u:u + 1])
            c2 = cos_sb[:, st * TS:(st + 1) * TS]
            s2 = sin_sb[:, st * TS:(st + 1) * TS]
            scrA = spool.tile([128, TS], BF16, tag="scrA", name="scrA")
            scrB = spool.tile([128, TS], BF16, tag="scrB", name="scrB")
            nc.vector.tensor_mul(scrA[0:64, :], tmp[64:128, :], s2[64:128, :])
            nc.vector.tensor_mul(scrA[64:128, :], tmp[0:64, :], s2[0:64, :])
            nc.vector.tensor_mul(scrB[:], tmp[:], c2)
            nc.vector.tensor_add(dst, scrB[:], scrA[:])

        # ---------------- S1 KV pass (k=7, v=8) -------------------------
        kps, vps = [None, None], [None, None]
        for st in (0, 1):
            kps[st] = ps.tile([128, TS], F32, tag="mm", bufs=4, name="kps")
            vps[st] = ps.tile([128, TS], F32, tag="mm", bufs=4, name="vps")
            for dc in range(DC):
                nc.tensor.matmul(kps[st][:], wqk_sb[:, dc, NQ, :],
                                 x_sb[st][:, dc, :],
                                 start=(dc == 0), stop=(dc == DC - 1))
                nc.tensor.matmul(vps[st][:], wqk_sb[:, dc, NQ + 1, :],
                                 x_sb[st][:, dc, :],
                                 start=(dc == 0), stop=(dc == DC - 1))
        for st in (0, 1):
            rope_drain(kps[st], NQ, st, blob[:, st * TS:(st + 1) * TS])
            vtmp = tpool.tile([128, TS], F32, tag="vtmp", bufs=2, name="vtmp")
            nc.scalar.activation(vtmp[:], vps[st][:],
                                 mybir.ActivationFunctionType.Identity,
                                 bias=bias_sb[:, NQ + 1:NQ + 2])
            for j in range(4):
                vp = ps.tile([128, TS], F32, tag="av", bufs=2, name="vp")
                nc.tensor.transpose(vp[:, 0:128],
                                    vtmp[:, j * 128:(j + 1) * 128],
                                    ident_sb[:])
                cl = 1024 + (4 * st + j) * 128
                nc.scalar.copy(blob[:, cl:cl + 128], vp[:, 0:128])

        # ---------------- pairwise K/V all-gather ------------------------
        nc.sync.dma_start(in_b[:], blob[:])
        nc.gpsimd.collective_compute(
            "AllGather",
            mybir.AluOpType.bypass,
            replica_groups=[[0, 1], [2, 3], [4, 5], [6, 7]],
            ins=[in_b.opt()],
            outs=[out_b.opt()],
        )
        for r in (0, 1):
            nc.sync.dma_start(kT_full[:, r::2, :], out_b[r, :, 0:8, :])
            nc.sync.dma_start(v_nat[:, r::2, :], out_b[r, :, 8:16, :])

        # ---------------- S1 Q passes (overlap the collective) -----------
        for st in (0, 1):
            for grp in (range(0, 4), range(4, NQ)):
                ups = [ps.tile([128, TS], F32, tag="mm", bufs=4,
                               name=f"up{u}") for u in grp]
                for dc in range(DC):
                    for i, u in enumerate(grp):
                        nc.tensor.matmul(ups[i][:], wqk_sb[:, dc, u, :],
                                         x_sb[st][:, dc, :],
                                         start=(dc == 0), stop=(dc == DC - 1))
                for i, u in enumerate(grp):
                    rope_drain(ups[i], u, st,
                               qaT[u][:, st * TS:(st + 1) * TS])

        # ---------------- S2 attention ----------------------------------
        for st in (0, 1):
            prof = W_ST[st]
            nsc = len(prof)
            cells = [(hq, sc) for hq in range(NQ) for sc in range(nsc)]
            pts = [None] * len(cells)

            def issue_scores(i, st=st, prof=prof, cells=cells, pts=pts):
                hq, sc = cells[i]
                o = (4 - prof[sc]) * 128
                sp = ps.tile([128, TS], F32, tag="mm", bufs=4, name="scps")
                nc.tensor.matmul(
                    sp[:, o:TS], kT_full[:, sc, :],
                    qaT[hq][:, st * TS + o:(st + 1) * TS],
                    start=True, stop=True)
                if sc >= 8 * st:
                    pp = ((sc - 8 * st) // 2) * 128
                    nc.vector.tensor_add(sp[:, pp:pp + 128],
                                         sp[:, pp:pp + 128],
                                         mask_sb[:, sc, :])
                pt = ppool.tile([128, TS], BF16, tag="pt", name="pt")
                nc.scalar.activation(pt[:, o:TS], sp[:, o:TS],
                                     mybir.ActivationFunctionType.Exp,
                                     scale=SCALE)
                pts[i] = pt

            LA = 4
            den_ps = av_ps = None
            for i0 in range(min(LA, len(cells))):
                issue_scores(i0)
            for i, (hq, sc) in enumerate(cells):
                if i + LA < len(cells):
                    issue_scores(i + LA)
                o = (4 - prof[sc]) * 128
                if sc == 0:
                    den_ps = ps.tile([128, TS], F32, tag="den", bufs=2,
                                     name="den")
                    av_ps = ps.tile([128, TS], F32, tag="av", bufs=2,
                                    name="av")
                stf, spf = (sc == 0), (sc == nsc - 1)
                pt = pts[i]
                nc.tensor.matmul(den_ps[:, o:TS], ones_sb[:], pt[:, o:TS],
                                 start=stf, stop=spf)
                nc.tensor.matmul(av_ps[:, o:TS], v_nat[:, sc, :],
                                 pt[:, o:TS], start=stf, stop=spf)
                pts[i] = None
                if spf:
                    recip = rpool.tile([128, TS], F32, tag="recip",
                                       name="recip")
                    nc.vector.reciprocal_approx_fast(recip[:], den_ps[:])
                    nc.vector.tensor_mul(
                        qaT[hq][:, st * TS:(st + 1) * TS],
                        av_ps[:], recip[:])

        # ---------------- S3 output projection ---------------------------
        for nt in range(D // TS):
            wo_t = wpool.tile([128, NQ, TS], BF16, tag="wo", name=f"wo{nt}")
            nc.sync.dma_start(wo_t[:], wo_d[:, :, nt * TS:(nt + 1) * TS])
            for tb0 in range(0, MYB, 2):
                yps = [ps.tile([128, TS], F32, tag="mm", bufs=4,
                               name=f"yp{j}") for j in range(2)]
                for u in range(NQ):
                    for j in range(2):
                        tbl = slice((tb0 + j) * 128, (tb0 + j + 1) * 128)
                        nc.tensor.matmul(yps[j][:], qaT[u][:, tbl],
                                         wo_t[:, u, :],
                                         start=(u == 0), stop=(u == NQ - 1))
                for j in range(2):
                    yout = ypool.tile([128, TS], BF16, tag="yo", name="yout")
                    if (tb0 + j + nt) % 2 == 0:
                        nc.scalar.copy(yout[:], yps[j][:])
                    else:
                        nc.vector.tensor_copy(yout[:], yps[j][:])
                    nc.sync.dma_start(
                        y_d[(tb0 + j) * 128:(tb0 + j + 1) * 128,
                            nt * TS:(nt + 1) * TS], yout[:])


def kernel(x, attn_mask, sin, cos, wq, wk, wv, wo, q_bias, k_bias, v_bias):
    x = np.asarray(x, np.float32)
    mask = np.asarray(attn_mask).astype(bool)
    sin = np.asarray(sin, np.float32)
    cos = np.asarray(cos, np.float32)
    wq = np.asarray(wq, np.float32)
    wk = np.asarray(wk, np.float32)
    wv = np.asarray(wv, np.float32)
    wo = np.asarray(wo, np.float32)
    q_bias = np.asarray(q_bias, np.float32).reshape(N, H)
    k_bias = np.asarray(k_bias, np.float32).reshape(KH, H)
    v_bias = np.asarray(v_bias, np.float32).reshape(KH, H)

    # causal-mask sanity: the kernel hardcodes the causal structure
    assert mask[0, 10, :11].all() and not mask[0, 10, 11:].any()

    BF = ml_dtypes.bfloat16
    xT = np.ascontiguousarray(x[0].T)                        # [D, T]
    c = cos[0].T                                             # [64, T]
    s = sin[0].T
    ident = np.eye(128, dtype=np.float32)
    ones128 = np.ones((128, 128), BF)

    # per-lane token index lists (even/odd 128-blocks)
    toks = {}
    for lane in (0, 1):
        toks[lane] = np.concatenate(
            [np.arange(b * 128, (b + 1) * 128) for b in range(lane, NB, 2)])

    # per-lane tensors
    xT16, cosT, sinT, maskC = {}, {}, {}, {}
    p_idx = np.arange(128)[:, None]
    j_idx = np.arange(128)[None, :]
    tri = np.where(p_idx > j_idx, MASKVAL, 0.0).astype(np.float32)
    full = np.full((128, 128), MASKVAL, np.float32)
    zero = np.zeros((128, 128), np.float32)
    for lane in (0, 1):
        tk = toks[lane]
        xl = xT[:, tk]                                       # [D, 1024]
        xT16[lane] = np.ascontiguousarray(
            xl.reshape(DC, 128, 2, TS).transpose(2, 1, 0, 3)).astype(BF)
        cc = np.concatenate([c[:, tk], c[:, tk]], 0)         # [128, 1024]
        ss = np.concatenate([s[:, tk], -s[:, tk]], 0)
        cosT[lane] = np.ascontiguousarray(cc).astype(BF)
        sinT[lane] = np.ascontiguousarray(ss).astype(BF)
        # mask content per key chunk sc (applied at block position
        # (sc - 8*st)//2 of the stripe): my block there is
        # b = sc + lane (sc even) or sc - 1 + lane (sc odd).
        mlist = []
        for sc in range(NB):
            if sc % 2 == lane:
                mlist.append(tri)          # diagonal block
            elif lane == 0:
                mlist.append(full)         # sc odd: b = sc-1 < sc
            else:
                mlist.append(zero)         # sc even: b = sc+1 > sc
        maskC[lane] = np.ascontiguousarray(
            np.stack(mlist, 1)).astype(BF)                   # [128, 16, 128]

    in_maps = []
    for cix in range(8):
        p = cix // 2
        lane = cix % 2
        qh = list(range(7 * p, 7 * p + 7))
        cols = [wq[:, h, :] for h in qh] + [wk[:, p, :], wv[:, p, :]]
        bcols = [q_bias[h] for h in qh] + [k_bias[p], v_bias[p]]
        wqk = np.stack(cols, axis=1)                         # [D, 9, 128]
        wqk16 = np.ascontiguousarray(
            wqk.reshape(DC, 128, NU, 128).transpose(1, 0, 2, 3)).astype(BF)
        biasT = np.stack(bcols, axis=1)                      # [128, 9]
        woT = np.ascontiguousarray(
            wo[qh].transpose(1, 0, 2)).astype(BF)            # [128, 7, D]
        in_maps.append({
            "xT16": xT16[lane], "wqk16": wqk16, "biasT": biasT,
            "cosT": cosT[lane], "sinT": sinT[lane], "ident": ident,
            "maskC": maskC[lane], "woT": woT, "ones": ones128,
        })

    nc = build_program()
    res = run_bass_kernel_spmd(nc, in_maps, list(range(8)), trace=_TRACE)
    if _TRACE and res.exec_time_ns is not None:
        print(f"HW exec time: {res.exec_time_ns} ns")
    y = np.zeros((T, D), np.float64)
    for cix in range(8):
        lane = cix % 2
        r = res.results[cix]["y"].astype(np.float64)         # [1024, D]
        for i, b in enumerate(range(lane, NB, 2)):
            y[b * 128:(b + 1) * 128] += r[i * 128:(i + 1) * 128]
    return y.reshape(B, T, D).astype(np.float32)
